# revision 7
# baseline (speedup 1.0000x reference)
"""Trainium2 Bass kernel: points-renderer compositor.

kernel(features, idx, zbuf, dist_xy) -> (4, 65, 256, 256) f32
  features: (4, 100000, 64) f32   packed per-batch feature table
  idx:      (4, 256, 256, 8) i64  packed point indices into (B*N, F) table
  zbuf:     (4, 256, 256, 8) f32
  dist_xy:  (4, 256, 256, 8) f32

Sharding: 8 cores = (batch b = c//2) x (y-half h = c%2, rows h*128..h*128+127).
Each core gets the full feature table (indices cross batches) plus its own
raster tiles, renders 128 rows, and the host reassembles the full output.

Per-core pipeline:
  w = min(exp(-dist/r^2), 0.99)              (ACT + DVE)
  contrib_k = w_k * prod_{k'<k}(1 - w_k')    (DVE, unrolled over K=8)
  depth = sum_k z_k w_k / max(sum_k w_k, 1e-9)
  for each 8-column block g (32 blocks):
    indirect-DMA gather 128x64 feature rows  (gpsimd SWDGE)
    multiply by contrib (f-broadcast), tree-reduce over k into ACC[y, x, f]
    PE-transpose ACC x-columns into ACCT[ch|y-half, y, x]
  DMA ACCT + depth to the (65, 128, 256) output
"""

import numpy as np

import concourse.tile as tile
from concourse import bacc, bass, mybir
from concourse.bass_utils import run_bass_kernel_spmd
from concourse.masks import make_identity

B, N, F = 4, 100000, 64
S, K = 256, 8
P = 128
NCORES = 8
NEG_INV_R2 = -16384.0  # -(S / (2*RADIUS))^2 = -1/r^2
F32 = mybir.dt.float32
I32 = mybir.dt.int32

_NC = None


def _build():
    nc = bacc.Bacc("TRN2", target_bir_lowering=False, debug=False)
    feats = nc.dram_tensor("features", [B * N, F], F32, kind="ExternalInput").ap()
    idxg_d = nc.dram_tensor("idxg", [P, 32, 64], I32, kind="ExternalInput").ap()
    distg_d = nc.dram_tensor("distg", [P, K, S], F32, kind="ExternalInput").ap()
    zg_d = nc.dram_tensor("zg", [P, K, S], F32, kind="ExternalInput").ap()
    outp = nc.dram_tensor("outp", [F + 1, P, S], F32, kind="ExternalOutput").ap()

    AL = mybir.AluOpType
    with tile.TileContext(nc) as tc:
        with tc.tile_pool(name="persist", bufs=1) as pp, \
             tc.tile_pool(name="psum", bufs=8, space="PSUM") as psp:
            idxg = pp.tile([P, 32, 64], I32)
            contrib = pp.tile([P, K, S], F32)
            # acct[p, yl, x]: p<64 -> (ch=p, y=yl); p>=64 -> (ch=p-64, y=64+yl)
            acct = pp.tile([P, 64, S], F32)
            depth = pp.tile([P, S], F32)
            ident = pp.tile([P, P], F32)

            nc.sync.dma_start(idxg[:], idxg_d[:])
            make_identity(nc, ident[:])

            with tc.tile_pool(name="phase_a", bufs=1) as pa:
                dist = pa.tile([P, K, S], F32)
                zw = pa.tile([P, K, S], F32)
                w = pa.tile([P, K, S], F32)
                trans = pa.tile([P, S], F32)
                om = pa.tile([P, S], F32)
                wsum = pa.tile([P, S], F32)
                zsum = pa.tile([P, S], F32)
                recip = pa.tile([P, S], F32)

                nc.sync.dma_start(dist[:], distg_d[:])
                nc.sync.dma_start(zw[:], zg_d[:])

                # w = min(exp(-dist/r^2), 0.99)
                nc.scalar.activation(
                    w[:], dist[:], mybir.ActivationFunctionType.Exp, scale=NEG_INV_R2
                )
                nc.vector.tensor_scalar_min(w[:], w[:], 0.99)

                # front-to-back compositing: contrib_k = w_k * prod_{k'<k}(1-w_k')
                nc.vector.tensor_copy(contrib[:, 0], w[:, 0])
                nc.vector.tensor_scalar(
                    out=trans[:], in0=w[:, 0], scalar1=-1.0, scalar2=1.0,
                    op0=AL.mult, op1=AL.add,
                )
                for k in range(1, K):
                    nc.vector.tensor_tensor(
                        out=contrib[:, k], in0=w[:, k], in1=trans[:], op=AL.mult
                    )
                    if k < K - 1:
                        nc.vector.tensor_scalar(
                            out=om[:], in0=w[:, k], scalar1=-1.0, scalar2=1.0,
                            op0=AL.mult, op1=AL.add,
                        )
                        nc.vector.tensor_tensor(
                            out=trans[:], in0=trans[:], in1=om[:], op=AL.mult
                        )

                # depth = (sum_k z_k w_k) / max(sum_k w_k, 1e-9)
                nc.vector.tensor_tensor(out=zw[:], in0=zw[:], in1=w[:], op=AL.mult)
                nc.vector.tensor_tensor(out=w[:, 0:4], in0=w[:, 0:4], in1=w[:, 4:8], op=AL.add)
                nc.vector.tensor_tensor(out=w[:, 0:2], in0=w[:, 0:2], in1=w[:, 2:4], op=AL.add)
                nc.vector.tensor_tensor(out=wsum[:], in0=w[:, 0], in1=w[:, 1], op=AL.add)
                nc.vector.tensor_tensor(out=zw[:, 0:4], in0=zw[:, 0:4], in1=zw[:, 4:8], op=AL.add)
                nc.vector.tensor_tensor(out=zw[:, 0:2], in0=zw[:, 0:2], in1=zw[:, 2:4], op=AL.add)
                nc.vector.tensor_tensor(out=zsum[:], in0=zw[:, 0], in1=zw[:, 1], op=AL.add)
                nc.vector.tensor_scalar_max(wsum[:], wsum[:], 1e-9)
                nc.vector.reciprocal(recip[:], wsum[:])
                nc.vector.tensor_tensor(out=depth[:], in0=zsum[:], in1=recip[:], op=AL.mult)

            nc.sync.dma_start(outp[F], depth[:])

            with tc.tile_pool(name="gather", bufs=2) as gp, \
                 tc.tile_pool(name="dup", bufs=2) as dp:
                for g in range(32):
                    gt = gp.tile([P, K, 8, F], F32, name="gt")
                    # gt[y, k, j, f] = feats[idxg[y, g, k*8+j], f]
                    # One gather instr per slot: HW SWDGE consumes exactly ONE
                    # offset per partition per instruction (desc = the
                    # partition's contiguous dst run); multi-offset or strided
                    # dst layouts mis-execute on HW (probe-verified).
                    gt2 = gt[:].rearrange("p k j f -> p (k j) f")
                    for s in range(64):
                        nc.gpsimd.indirect_dma_start(
                            out=gt2[:, s],
                            out_offset=None, in_=feats[:],
                            in_offset=bass.IndirectOffsetOnAxis(
                                ap=idxg[:, g, s:s + 1], axis=0
                            ),
                        )
                    cb = contrib[:, :, g * 8:(g + 1) * 8, None].to_broadcast([P, K, 8, F])
                    nc.vector.tensor_tensor(out=gt[:], in0=gt[:], in1=cb, op=AL.mult)
                    nc.vector.tensor_tensor(out=gt[:, 0:4], in0=gt[:, 0:4], in1=gt[:, 4:8], op=AL.add)
                    nc.vector.tensor_tensor(out=gt[:, 0:2], in0=gt[:, 0:2], in1=gt[:, 2:4], op=AL.add)
                    # k-reduced block written twice (dup axis) so the per-x
                    # transpose lhsT is a contiguous 128-wide stationary AP
                    # filling all 128 PSUM partitions -> both acct copies are
                    # partition-shift-free.
                    dup = dp.tile([P, 8, 2, F], F32, name="dup")
                    nc.vector.tensor_tensor(out=dup[:, :, 0, :], in0=gt[:, 0], in1=gt[:, 1], op=AL.add)
                    nc.vector.tensor_tensor(out=dup[:, :, 1, :], in0=gt[:, 0], in1=gt[:, 1], op=AL.add)

                    for xi in range(8):
                        x = g * 8 + xi
                        ps = psp.tile([P, P], F32, name="ps")
                        # ps[q, y] = block[y, x, q % 64]
                        nc.tensor.transpose(
                            out=ps[:], in_=dup[:, xi], identity=ident[:]
                        )
                        nc.vector.tensor_copy(acct[0:64, :, x], ps[0:64, 0:64])
                        nc.vector.tensor_copy(acct[64:128, :, x], ps[64:128, 64:128])

            nc.sync.dma_start(outp[0:F, 0:64, :], acct[0:64])
            nc.sync.dma_start(outp[0:F, 64:128, :], acct[64:128])

    nc.compile()
    return nc


def _get_nc():
    global _NC
    if _NC is None:
        _NC = _build()
    return _NC


def _prep(features, idx, zbuf, dist_xy):
    feats_packed = np.ascontiguousarray(
        np.asarray(features, dtype=np.float32).reshape(B * N, F)
    )
    idx32 = np.asarray(idx).astype(np.int32)
    zb = np.asarray(zbuf, dtype=np.float32)
    dxy = np.asarray(dist_xy, dtype=np.float32)
    in_maps = []
    for c in range(NCORES):
        b, h = divmod(c, 2)
        y0 = h * P
        blk = idx32[b, y0:y0 + P]  # (128, 256, 8) [y, x, k]
        idxg = np.ascontiguousarray(
            blk.reshape(P, 32, 8, K).transpose(0, 1, 3, 2).reshape(P, 32, 64)
        )
        distg = np.ascontiguousarray(dxy[b, y0:y0 + P].transpose(0, 2, 1))
        zg = np.ascontiguousarray(zb[b, y0:y0 + P].transpose(0, 2, 1))
        in_maps.append(
            {"features": feats_packed, "idxg": idxg, "distg": distg, "zg": zg}
        )
    return in_maps


def _run(in_maps, trace=False):
    return run_bass_kernel_spmd(
        _get_nc(), in_maps, core_ids=list(range(NCORES)), trace=trace
    )


def kernel(features, idx, zbuf, dist_xy):
    res = _run(_prep(features, idx, zbuf, dist_xy))
    out = np.empty((B, F + 1, S, S), dtype=np.float32)
    for c in range(NCORES):
        b, h = divmod(c, 2)
        out[b, :, h * P:(h + 1) * P, :] = res.results[c]["outp"]
    return out
